# revision 10
# baseline (speedup 1.0000x reference)
"""Trainium2 Bass kernel for nn_Blur: upfirdn2d(up=2, k=4x4 separable binomial).

Math: per (n,c) plane X [128,128] the output is out = A.T @ X @ A with
A [128,255] the 1D polyphase upsampling matrix (2 taps per output row).
Both 1D passes are 2-tap polyphase filters; with v symmetric, every output
row/col is (a + r*b) or (r*a + b) up to a global scale v1^2 (r = v3/v1).
The global scale is folded into the input on the HOST (imgs * v1^2), so the
device runs a pure unit-coefficient chain of fused scalar_tensor_tensor ops.

Device pipeline (pure DVE + DMA; PE/ACT idle; all DMAs SWDGE on one queue):
  - Input DMA: imgs -> x[g, win, h, w] bf16, g on partitions, each partition
    reads its planes CONTIGUOUSLY (line-rate); SWDGE casts fp32->bf16.
    x has a zeroed pad row h=128 per window. The whole 8MB input lives in
    SBUF (it IS the per-core input in bf16); window 1's load is issued after
    window 0's first block so the read burst interleaves into the output
    stream instead of colliding with it up front.
  - Per 64-row output block:
    H-pass on DVE (bf16 2x_1P mode: innermost w step-1/even/4B-aligned,
    single-port so it never blocks SWDGE descriptor gen):
      s[g, 2t,   w] = x[g, r0+t, w] + r * x[g, r0+t+1, w]
      s[g, 2t+1, w] = r * x[g, r0+t, w] + x[g, r0+t+1, w]
    s bf16 [128, 64, 130] with zeroed pad col w=128, double-buffered.
    W-pass on DVE (bf16 out, 1x: strided x-interleaved writes):
      o[g, y, 0:255:2] = s[g,y,j] + r*s[g,y,j+1]   (pad col covers x=254)
      o[g, y, 1:254:2] = r*s[g,y,j] + s[g,y,j+1]
  - Output DMA (SWDGE, casts bf16->fp32 on store): [g, 64y, 255x] ->
    per-partition contiguous ~65KB runs in DRAM (big-transfer BW regime).
Sharding: pure data parallel over batch, 2 images (256 planes) per core.
"""

import math

import numpy as np

import concourse.bacc as bacc
import concourse.mybir as mybir
import concourse.tile as tile
from concourse.bass_utils import run_bass_kernel_spmd

N_CORES = 8
N, C, H, W = 16, 128, 128, 128
HO = 2 * H - 1  # 255
PLANES_PER_CORE = (N // N_CORES) * C  # 256
WINDOW = 128  # planes per window (= output DMA partition span)
BLEN = 64  # output rows per block (staging tile / DMA)
XH = 130  # x tile h-dim per window: 0..127 data, 128 zero, 129 align pad
SW = 130  # s tile w-dim: 0..127 data, 128 zero pad, 129 align pad
DT = mybir.dt.float32
BF = mybir.dt.bfloat16


def _taps_from_kernel(kernel2d: np.ndarray) -> np.ndarray:
    """Recover the 1D taps v (kernel2d == outer(v, v))."""
    k = np.asarray(kernel2d, dtype=np.float64)
    assert k.shape == (4, 4)
    v0 = math.sqrt(k[0, 0])
    v = k[0] / v0
    assert np.allclose(np.outer(v, v), k, rtol=1e-6), "kernel is not rank-1"
    assert abs(v[0] - v[3]) < 1e-12 and abs(v[1] - v[2]) < 1e-12, (
        "kernel taps not symmetric"
    )
    return v


def _build_amat(v: np.ndarray) -> np.ndarray:
    """A' = v1 * A, where A [128, 255] maps input rows to upsampled rows.

    (Unused by the DVE-only device program, but kept as the host-side
    reference for the polyphase structure and for the amat input tensor.)
    """
    A = np.zeros((H, HO), dtype=np.float64)
    for y in range(HO):
        if y % 2 == 0:
            r = y // 2
            A[r, y] += v[1]
            if r + 1 < H:
                A[r + 1, y] += v[3]
        else:
            A[(y - 1) // 2, y] += v[0]
            A[(y + 1) // 2, y] += v[2]
    return (v[1] * A).astype(np.float32)


def _chunks(total: int, step: int):
    return [(s, min(step, total - s)) for s in range(0, total, step)]


def _build_bass(ratio: float, loop: int = 1, internal_out: bool = False):
    """Trace + compile the per-core Tile program. ratio = v3/v1."""
    nc = bacc.Bacc(
        "TRN2", target_bir_lowering=False, debug=False, num_devices=N_CORES
    )
    amat_d = nc.dram_tensor("amat", [H, HO], DT, kind="ExternalInput")
    if internal_out:
        # timing-only build: no big tensors cross the host link
        imgs_d = nc.dram_tensor("imgs_t", [PLANES_PER_CORE, H, W], DT)
        out_d = nc.dram_tensor("out", [PLANES_PER_CORE, HO, HO], DT)
        done_d = nc.dram_tensor("done", [1, 4], DT, kind="ExternalOutput")
    else:
        imgs_d = nc.dram_tensor(
            "imgs", [PLANES_PER_CORE, H, W], DT, kind="ExternalInput"
        )
        out_d = nc.dram_tensor(
            "out", [PLANES_PER_CORE, HO, HO], DT, kind="ExternalOutput"
        )
        done_d = None

    mult = mybir.AluOpType.mult
    add = mybir.AluOpType.add
    n_win = PLANES_PER_CORE // WINDOW  # 2

    with tile.TileContext(nc) as tc:
        with (
            tc.tile_pool(name="const", bufs=1) as const_pool,
            tc.tile_pool(name="xin", bufs=1) as in_pool,
            tc.tile_pool(name="sblk", bufs=1) as s_pool,
            tc.tile_pool(name="outp", bufs=2) as out_pool,
        ):
            a1 = const_pool.tile([H, 4], DT, tag="a1", name="a1")
            nc.sync.dma_start(a1[:], amat_d[:, 0:4])

            # whole per-core input in SBUF (bf16), zeroed pad row per window
            x = in_pool.tile([128, n_win, XH, W], BF, tag="x", name="x")
            s_tiles = [
                s_pool.tile([128, BLEN, SW], BF, tag=f"s{i}", name=f"s{i}")
                for i in range(2)
            ]
            nc.vector.memset(x[:, :, 128, :], 0.0)
            for st in s_tiles:
                nc.vector.memset(st[:, :, 128], 0.0)

            def load_window(win):
                g0 = win * WINDOW
                for k in range(4):
                    nc.gpsimd.dma_start(
                        x[:, win, 32 * k : 32 * (k + 1), :],
                        imgs_d[g0 : g0 + WINDOW][:, 32 * k : 32 * (k + 1), :],
                    )

            def block_body(win, g0, y0, blen, s):
                ne = (blen + 1) // 2  # even-y rows in this block
                no = blen // 2  # odd-y rows
                r0 = y0 // 2
                # H-pass: even y = y0+2t: x[r0+t] + r*x[r0+t+1]
                nc.vector.scalar_tensor_tensor(
                    s[:, 0 : 2 * ne : 2, 0:W],
                    x[:, win, r0 + 1 : r0 + 1 + ne, :], ratio,
                    x[:, win, r0 : r0 + ne, :],
                    op0=mult, op1=add,
                )
                # odd y = y0+2t+1: r*x[r0+t] + x[r0+t+1]
                nc.vector.scalar_tensor_tensor(
                    s[:, 1 : 2 * no : 2, 0:W],
                    x[:, win, r0 : r0 + no, :], ratio,
                    x[:, win, r0 + 1 : r0 + 1 + no, :],
                    op0=mult, op1=add,
                )
                # W-pass into bf16 staging
                o = out_pool.tile([128, BLEN, HO], BF, tag="o", name="o")
                # x = 2j   (j=0..127):   S[j] + r*S[j+1]  (S[128]=0)
                # x = 2j+1 (j=0..126): r*S[j] +   S[j+1]
                nc.vector.scalar_tensor_tensor(
                    o[:, 0:blen, 0:255:2],
                    s[:, 0:blen, 1:129], ratio, s[:, 0:blen, 0:128],
                    op0=mult, op1=add,
                )
                nc.vector.scalar_tensor_tensor(
                    o[:, 0:blen, 1:254:2],
                    s[:, 0:blen, 0:127], ratio, s[:, 0:blen, 1:128],
                    op0=mult, op1=add,
                )
                # SWDGE cast bf16 -> fp32 on store; ~65KB runs per partition
                dst = out_d[g0 : g0 + WINDOW]
                nc.gpsimd.dma_start(
                    dst[:, y0 : y0 + blen, :],
                    o[:, 0:blen, :],
                )

            def full_body():
                load_window(0)
                bi = 0
                for win in range(n_win):
                    g0 = win * WINDOW
                    for y0, blen in _chunks(HO, BLEN):
                        block_body(win, g0, y0, blen, s_tiles[bi % 2])
                        bi += 1
                        # prefetch window 1 between the first blocks so its
                        # read burst interleaves into the output stream
                        if win == 0 and bi == 1:
                            load_window(1)

            if loop == 1:
                full_body()
            else:
                with tc.For_i(0, loop) as _:
                    full_body()

            if done_d is not None:
                nc.sync.dma_start(done_d[:], a1[0:1, 0:4])

    nc.compile()
    return nc


_CACHE: dict = {}


def _get_bass(kernel2d: np.ndarray):
    key = np.asarray(kernel2d, dtype=np.float32).tobytes()
    if key not in _CACHE:
        v = _taps_from_kernel(kernel2d)
        amat = _build_amat(v)
        ratio = float(v[3] / v[1])
        scale = float(v[1] * v[1])
        _CACHE[key] = (_build_bass(ratio), amat, scale)
    return _CACHE[key]


def run(imgs: np.ndarray, kernel: np.ndarray, **spmd_kwargs):
    """Run on 8 NeuronCores; returns (full_output, BassKernelResults)."""
    imgs = np.asarray(imgs, dtype=np.float32)
    assert imgs.shape == (N, C, H, W)
    nc, amat, scale = _get_bass(kernel)
    # the device runs a unit-coefficient 2-tap chain; fold the global
    # v1^2 scale into the input here (linear, so exactly equivalent)
    imgs = np.ascontiguousarray(imgs * scale)

    per = N // N_CORES
    in_maps = [
        {
            "imgs": imgs[i * per : (i + 1) * per].reshape(
                PLANES_PER_CORE, H, W
            ),
            "amat": amat,
        }
        for i in range(N_CORES)
    ]
    res = run_bass_kernel_spmd(nc, in_maps, list(range(N_CORES)), **spmd_kwargs)
    out = np.concatenate(
        [r["out"].reshape(per, C, HO, HO) for r in res.results], axis=0
    )
    return out, res


def kernel(imgs: np.ndarray, kernel: np.ndarray) -> np.ndarray:
    out, _ = run(imgs, kernel)
    return out


# revision 12
# speedup vs baseline: 1.2479x; 1.2479x over previous
"""Trainium2 Bass kernel for nn_Blur: upfirdn2d(up=2, k=4x4 separable binomial).

Math: per (n,c) plane X [128,128] the output is out = A.T @ X @ A with
A [128,255] the 1D polyphase upsampling matrix (2 taps per output row).
Both 1D passes are 2-tap polyphase filters; with v symmetric, every output
row/col is (a + r*b) or (r*a + b) up to a global scale v1^2 (r = v3/v1).
The global scale is folded into the input on the HOST (imgs * v1^2), so the
device runs a pure unit-coefficient chain of fused scalar_tensor_tensor ops.

Device pipeline (pure DVE + DMA; PE/ACT idle; all DMAs SWDGE on one queue):
  - Input DMA: imgs -> x[g, win, h, w] bf16, g on partitions, each partition
    reads its planes CONTIGUOUSLY (line-rate); SWDGE casts fp32->bf16.
    x has a zeroed pad row h=128 per window. The whole 8MB input lives in
    SBUF (it IS the per-core input in bf16); window 1's load is issued after
    window 0's first block so the read burst interleaves into the output
    stream instead of colliding with it up front.
  - Per 64-row output block:
    H-pass on DVE (bf16 2x_1P mode: innermost w step-1/even/4B-aligned,
    single-port so it never blocks SWDGE descriptor gen):
      s[g, 2t,   w] = x[g, r0+t, w] + r * x[g, r0+t+1, w]
      s[g, 2t+1, w] = r * x[g, r0+t, w] + x[g, r0+t+1, w]
    s bf16 [128, 64, 130] with zeroed pad col w=128, double-buffered.
    W-pass on DVE (bf16 out, 1x: strided x-interleaved writes):
      o[g, y, 0:255:2] = s[g,y,j] + r*s[g,y,j+1]   (pad col covers x=254)
      o[g, y, 1:254:2] = r*s[g,y,j] + s[g,y,j+1]
  - Output DMA (SWDGE, casts bf16->fp32 on store): [g, 64y, 255x] ->
    per-partition contiguous ~65KB runs in DRAM (big-transfer BW regime).
Sharding: pure data parallel over batch, 2 images (256 planes) per core.
"""

import math

import numpy as np

import concourse.bacc as bacc
import concourse.mybir as mybir
import concourse.tile as tile
from concourse.bass_utils import run_bass_kernel_spmd

N_CORES = 8
N, C, H, W = 16, 128, 128, 128
HO = 2 * H - 1  # 255
PLANES_PER_CORE = (N // N_CORES) * C  # 256
WINDOW = 128  # planes per window (= output DMA partition span)
BLEN = 64  # output rows per block (staging tile / DMA)
XH = 130  # x tile h-dim per window: 0..127 data, 128 zero, 129 align pad
SW = 130  # s tile w-dim: 0..127 data, 128 zero pad, 129 align pad
SPLIT = 48  # rows per block on the SWDGE cast ring; rest go fp32 on sync
DT = mybir.dt.float32
BF = mybir.dt.bfloat16


def _taps_from_kernel(kernel2d: np.ndarray) -> np.ndarray:
    """Recover the 1D taps v (kernel2d == outer(v, v))."""
    k = np.asarray(kernel2d, dtype=np.float64)
    assert k.shape == (4, 4)
    v0 = math.sqrt(k[0, 0])
    v = k[0] / v0
    assert np.allclose(np.outer(v, v), k, rtol=1e-6), "kernel is not rank-1"
    assert abs(v[0] - v[3]) < 1e-12 and abs(v[1] - v[2]) < 1e-12, (
        "kernel taps not symmetric"
    )
    return v


def _build_amat(v: np.ndarray) -> np.ndarray:
    """A' = v1 * A, where A [128, 255] maps input rows to upsampled rows.

    (Unused by the DVE-only device program, but kept as the host-side
    reference for the polyphase structure and for the amat input tensor.)
    """
    A = np.zeros((H, HO), dtype=np.float64)
    for y in range(HO):
        if y % 2 == 0:
            r = y // 2
            A[r, y] += v[1]
            if r + 1 < H:
                A[r + 1, y] += v[3]
        else:
            A[(y - 1) // 2, y] += v[0]
            A[(y + 1) // 2, y] += v[2]
    return (v[1] * A).astype(np.float32)


def _chunks(total: int, step: int):
    return [(s, min(step, total - s)) for s in range(0, total, step)]


def _build_bass(ratio: float, loop: int = 1, internal_out: bool = False):
    """Trace + compile the per-core Tile program. ratio = v3/v1."""
    nc = bacc.Bacc(
        "TRN2", target_bir_lowering=False, debug=False, num_devices=N_CORES
    )
    amat_d = nc.dram_tensor("amat", [H, HO], DT, kind="ExternalInput")
    if internal_out:
        # timing-only build: no big tensors cross the host link
        imgs_d = nc.dram_tensor("imgs_t", [PLANES_PER_CORE, H, W], DT)
        out_d = nc.dram_tensor("out", [PLANES_PER_CORE, HO, HO], DT)
        done_d = nc.dram_tensor("done", [1, 4], DT, kind="ExternalOutput")
    else:
        imgs_d = nc.dram_tensor(
            "imgs", [PLANES_PER_CORE, H, W], DT, kind="ExternalInput"
        )
        out_d = nc.dram_tensor(
            "out", [PLANES_PER_CORE, HO, HO], DT, kind="ExternalOutput"
        )
        done_d = None

    mult = mybir.AluOpType.mult
    add = mybir.AluOpType.add
    n_win = PLANES_PER_CORE // WINDOW  # 2

    with tile.TileContext(nc) as tc:
        with (
            tc.tile_pool(name="const", bufs=1) as const_pool,
            tc.tile_pool(name="xin", bufs=1) as in_pool,
            tc.tile_pool(name="sblk", bufs=1) as s_pool,
            tc.tile_pool(name="outp", bufs=2) as out_pool,
        ):
            a1 = const_pool.tile([H, 4], DT, tag="a1", name="a1")
            nc.sync.dma_start(a1[:], amat_d[:, 0:4])

            # whole per-core input in SBUF (bf16), zeroed pad row per window
            x = in_pool.tile([128, n_win, XH, W], BF, tag="x", name="x")
            s_tiles = [
                s_pool.tile([128, BLEN, SW], BF, tag=f"s{i}", name=f"s{i}")
                for i in range(2)
            ]
            nc.vector.memset(x[:, :, 128, :], 0.0)
            for st in s_tiles:
                nc.vector.memset(st[:, :, 128], 0.0)

            def load_window(win):
                g0 = win * WINDOW
                for k in range(4):
                    nc.gpsimd.dma_start(
                        x[:, win, 32 * k : 32 * (k + 1), :],
                        imgs_d[g0 : g0 + WINDOW][:, 32 * k : 32 * (k + 1), :],
                    )

            def block_body(win, g0, y0, blen, s):
                ne = (blen + 1) // 2  # even-y rows in this block
                no = blen // 2  # odd-y rows
                r0 = y0 // 2
                # H-pass: even y = y0+2t: x[r0+t] + r*x[r0+t+1]
                nc.vector.scalar_tensor_tensor(
                    s[:, 0 : 2 * ne : 2, 0:W],
                    x[:, win, r0 + 1 : r0 + 1 + ne, :], ratio,
                    x[:, win, r0 : r0 + ne, :],
                    op0=mult, op1=add,
                )
                # odd y = y0+2t+1: r*x[r0+t] + x[r0+t+1]
                nc.vector.scalar_tensor_tensor(
                    s[:, 1 : 2 * no : 2, 0:W],
                    x[:, win, r0 : r0 + no, :], ratio,
                    x[:, win, r0 + 1 : r0 + 1 + no, :],
                    op0=mult, op1=add,
                )
                # W-pass, split across two staging tiles / DMA rings:
                # rows 0..SPLIT-1 -> bf16, SWDGE cast-DMA (qPoolDynamic);
                # rows SPLIT..blen-1 -> fp32, sync HWDGE DMA (qSPDynamicHW).
                # Two parallel FIFOs carry the write stream.
                lo = min(SPLIT, blen)
                hi = blen - lo
                o = out_pool.tile([128, SPLIT, HO], BF, tag="o", name="o")
                oh = out_pool.tile([128, BLEN - SPLIT, HO], DT, tag="oh", name="oh")
                dst = out_d[g0 : g0 + WINDOW]
                for dst_t, dy0, n, src_rows in (
                    (o, 0, lo, slice(0, lo)),
                    (oh, lo, hi, slice(lo, blen)),
                ):
                    if n == 0:
                        continue
                    # x = 2j   (j=0..127):   S[j] + r*S[j+1]  (S[128]=0)
                    # x = 2j+1 (j=0..126): r*S[j] +   S[j+1]
                    nc.vector.scalar_tensor_tensor(
                        dst_t[:, 0:n, 0:255:2],
                        s[:, src_rows, 1:129], ratio, s[:, src_rows, 0:128],
                        op0=mult, op1=add,
                    )
                    nc.vector.scalar_tensor_tensor(
                        dst_t[:, 0:n, 1:254:2],
                        s[:, src_rows, 0:127], ratio, s[:, src_rows, 1:128],
                        op0=mult, op1=add,
                    )
                nc.gpsimd.dma_start(
                    dst[:, y0 : y0 + lo, :], o[:, 0:lo, :]
                )
                if hi:
                    nc.sync.dma_start(
                        dst[:, y0 + lo : y0 + blen, :], oh[:, 0:hi, :]
                    )

            def full_body():
                # all input up front: interleaving the read burst into the
                # output stream measured ~60us WORSE (R/W mixing penalty)
                load_window(0)
                load_window(1)
                bi = 0
                for win in range(n_win):
                    g0 = win * WINDOW
                    for y0, blen in _chunks(HO, BLEN):
                        block_body(win, g0, y0, blen, s_tiles[bi % 2])
                        bi += 1

            if loop == 1:
                full_body()
            else:
                with tc.For_i(0, loop) as _:
                    full_body()

            if done_d is not None:
                nc.sync.dma_start(done_d[:], a1[0:1, 0:4])

    nc.compile()
    return nc


_CACHE: dict = {}


def _get_bass(kernel2d: np.ndarray):
    key = np.asarray(kernel2d, dtype=np.float32).tobytes()
    if key not in _CACHE:
        v = _taps_from_kernel(kernel2d)
        amat = _build_amat(v)
        ratio = float(v[3] / v[1])
        scale = float(v[1] * v[1])
        _CACHE[key] = (_build_bass(ratio), amat, scale)
    return _CACHE[key]


def run(imgs: np.ndarray, kernel: np.ndarray, **spmd_kwargs):
    """Run on 8 NeuronCores; returns (full_output, BassKernelResults)."""
    imgs = np.asarray(imgs, dtype=np.float32)
    assert imgs.shape == (N, C, H, W)
    nc, amat, scale = _get_bass(kernel)
    # the device runs a unit-coefficient 2-tap chain; fold the global
    # v1^2 scale into the input here (linear, so exactly equivalent)
    imgs = np.ascontiguousarray(imgs * scale)

    per = N // N_CORES
    in_maps = [
        {
            "imgs": imgs[i * per : (i + 1) * per].reshape(
                PLANES_PER_CORE, H, W
            ),
            "amat": amat,
        }
        for i in range(N_CORES)
    ]
    res = run_bass_kernel_spmd(nc, in_maps, list(range(N_CORES)), **spmd_kwargs)
    out = np.concatenate(
        [r["out"].reshape(per, C, HO, HO) for r in res.results], axis=0
    )
    return out, res


def kernel(imgs: np.ndarray, kernel: np.ndarray) -> np.ndarray:
    out, _ = run(imgs, kernel)
    return out
